# revision 7
# baseline (speedup 1.0000x reference)
"""DIEN (nn_DIEN_32049045963136) Trainium2 Bass kernel.

Strategy: data-parallel over batch (1024 -> 128 rows/core on 8 cores).
Per core:
  - embedding gathers via per-timestep indirect DMAs (1 idx/partition),
    tables pre-cast to bf16 on host
  - behT/hsT live in one combined bf16 tile `combo` [128, 201*128]:
      rows 0:64  = hsT_t  at slot t+1   (slot 0 = h_init zeros)
      rows 64:128= behT_t at slot t
    so aux-MLP rhs [H+D,128] chunks come for free
  - GRU/AUGRU in transposed layout, xp projections computed per-step by
    matmuls straight into PSUM (no xp storage), biases folded into the
    activation instruction's per-partition bias
  - attention scores/softmax in B-layout from DMA-transposed hidden states
  - AUGRU attention scale via ones-matmul partition-broadcast of att row
  - outputs: final AUGRU hidden (transposed), user profile rows, aux-MLP
    click probabilities; tiny BN+3-layer head over [1024,96] runs on host
    (needs cross-core batch statistics)
"""
import numpy as np
import ml_dtypes

B, T, E = 1024, 200, 32
D = 2 * E       # 64
H = D           # 64
P_EMB = E       # 32
NCORES = 8
BC = B // NCORES  # 128 rows per core
NPAIR = (T - 1) * BC  # aux pairs per core

_BF16 = ml_dtypes.bfloat16

_STATE = {}


def _build():
    import concourse.bass as bass
    import concourse.tile as tile
    from concourse import bacc, mybir
    from concourse.bass import IndirectOffsetOnAxis, ts
    from concourse.masks import make_identity

    f32 = mybir.dt.float32
    bf16 = mybir.dt.bfloat16
    i32 = mybir.dt.int32
    AF = mybir.ActivationFunctionType
    OP = mybir.AluOpType

    nc = bacc.Bacc(None)

    # ---- inputs (per-core shapes) ----
    item_t = nc.dram_tensor("item_t", [1_000_000, E], bf16, kind="ExternalInput")
    cate_t = nc.dram_tensor("cate_t", [100_000, E], bf16, kind="ExternalInput")
    user_t = nc.dram_tensor("user_t", [100_000, E], f32, kind="ExternalInput")
    iidx_t = nc.dram_tensor("iidx", [BC, T], i32, kind="ExternalInput")
    cidx_t = nc.dram_tensor("cidx", [BC, T], i32, kind="ExternalInput")
    tidx_t = nc.dram_tensor("tidx", [BC, 1], i32, kind="ExternalInput")
    tcidx_t = nc.dram_tensor("tcidx", [BC, 1], i32, kind="ExternalInput")
    uidx_t = nc.dram_tensor("uidx", [BC, 1], i32, kind="ExternalInput")
    gwx_t = nc.dram_tensor("gwx", [D, 3 * H], bf16, kind="ExternalInput")
    gwh_t = nc.dram_tensor("gwh", [H, 3 * H], bf16, kind="ExternalInput")
    gbzr_t = nc.dram_tensor("gbzr", [2 * H, 1], f32, kind="ExternalInput")
    gbn_t = nc.dram_tensor("gbn", [H, 1], f32, kind="ExternalInput")
    awx_t = nc.dram_tensor("awx", [H + 1, 3 * H], bf16, kind="ExternalInput")
    awh_t = nc.dram_tensor("awh", [H, 3 * H], bf16, kind="ExternalInput")
    xw1_t = nc.dram_tensor("xw1", [H + D, 100], bf16, kind="ExternalInput")
    xb1_t = nc.dram_tensor("xb1", [100, 1], f32, kind="ExternalInput")
    xw2_t = nc.dram_tensor("xw2", [100, 50], bf16, kind="ExternalInput")
    xb2_t = nc.dram_tensor("xb2", [50, 1], f32, kind="ExternalInput")
    xw3d_t = nc.dram_tensor("xw3d", [50, 1], bf16, kind="ExternalInput")
    xb3d_t = nc.dram_tensor("xb3d", [1, 1], f32, kind="ExternalInput")

    # ---- outputs ----
    haug_o = nc.dram_tensor("haug", [BC, H], f32, kind="ExternalOutput")
    userp_o = nc.dram_tensor("userp", [BC, E], f32, kind="ExternalOutput")
    auxp_o = nc.dram_tensor("auxp", [1, NPAIR], f32, kind="ExternalOutput")

    NSLOT = T + 1  # combo slots

    with tile.TileContext(nc) as tc:
        with (
            tc.tile_pool(name="big", bufs=1) as big,
            tc.tile_pool(name="small", bufs=1) as small,
            tc.tile_pool(name="work", bufs=2) as work,
            tc.tile_pool(name="psum2", bufs=2, space="PSUM") as pp,
            tc.tile_pool(name="psum1", bufs=1, space="PSUM") as pp1,
        ):
            # --- load indices & weights ---
            idx_i = small.tile([BC, T], i32)
            idx_c = small.tile([BC, T], i32)
            idx_tgt = small.tile([BC, 1], i32)
            idx_tct = small.tile([BC, 1], i32)
            idx_usr = small.tile([BC, 1], i32)
            nc.sync.dma_start(idx_i[:], iidx_t[:])
            nc.sync.dma_start(idx_c[:], cidx_t[:])
            nc.sync.dma_start(idx_tgt[:], tidx_t[:])
            nc.sync.dma_start(idx_tct[:], tcidx_t[:])
            nc.sync.dma_start(idx_usr[:], uidx_t[:])

            gwx_full = small.tile([128, 3 * H], bf16)
            gwx = gwx_full[D:128, :]
            gwh = small.tile([H, 3 * H], bf16)
            gbzr = small.tile([2 * H, 1], f32)
            gbn = small.tile([H, 1], f32)
            awx = small.tile([H + 1, 3 * H], bf16)
            awh = small.tile([H, 3 * H], bf16)
            xw1 = small.tile([H + D, 100], bf16)
            xb1 = small.tile([100, 1], f32)
            xw2 = small.tile([100, 50], bf16)
            xb2 = small.tile([50, 1], f32)
            xw3d = small.tile([50, 1], bf16)
            xb3d = small.tile([1, 1], f32)
            nc.sync.dma_start(gwx, gwx_t[:])
            for sb_t, dr_t in (
                (gwh, gwh_t), (gbzr, gbzr_t), (gbn, gbn_t),
                (awx, awx_t), (awh, awh_t),
                (xw1, xw1_t), (xb1, xb1_t), (xw2, xw2_t), (xb2, xb2_t),
                (xw3d, xw3d_t), (xb3d, xb3d_t),
            ):
                nc.sync.dma_start(sb_t[:], dr_t[:])


            ident = small.tile([128, 128], bf16)
            make_identity(nc, ident[:])

            # --- target / user gathers ---
            targ = small.tile([BC, D], bf16)
            nc.gpsimd.indirect_dma_start(
                out=targ[:, 0:E], out_offset=None, in_=item_t[:],
                in_offset=IndirectOffsetOnAxis(ap=idx_tgt[:, 0:1], axis=0))
            nc.gpsimd.indirect_dma_start(
                out=targ[:, E:D], out_offset=None, in_=cate_t[:],
                in_offset=IndirectOffsetOnAxis(ap=idx_tct[:, 0:1], axis=0))
            usr = small.tile([BC, E], f32)
            nc.gpsimd.indirect_dma_start(
                out=usr[:], out_offset=None, in_=user_t[:],
                in_offset=IndirectOffsetOnAxis(ap=idx_usr[:, 0:1], axis=0))
            usr2 = small.tile([BC, E], f32)
            nc.vector.tensor_copy(usr2[:], usr[:])
            nc.sync.dma_start(userp_o[:], usr2[:])

            # --- behavior gathers (per-t, 1 idx/partition) ---
            beh = big.tile([BC, T * D], bf16)
            for t in range(T):
                nc.gpsimd.indirect_dma_start(
                    out=beh[:, t * D : t * D + E], out_offset=None, in_=item_t[:],
                    in_offset=IndirectOffsetOnAxis(ap=idx_i[:, t : t + 1], axis=0))
                nc.gpsimd.indirect_dma_start(
                    out=beh[:, t * D + E : (t + 1) * D], out_offset=None,
                    in_=cate_t[:],
                    in_offset=IndirectOffsetOnAxis(ap=idx_c[:, t : t + 1], axis=0))

            # --- combo tile: rows 0:64 hsT (slot t+1), rows 64:128 behT (slot t)
            combo = big.tile([128, NSLOT * BC], bf16)
            for t in range(T):
                p_bt = pp1.tile([D, BC], bf16, tag="tr")
                nc.tensor.transpose(out=p_bt[:], in_=beh[:, t * D : (t + 1) * D],
                                    identity=ident[:])
                nc.scalar.copy(combo[H:128, ts(t, BC)], p_bt[:])
            nc.vector.memset(combo[0:H, ts(0, BC)], 0.0)

            # --- GRU over t ---
            h_prev = work.tile([H, BC], f32, tag="hstate")
            nc.vector.memset(h_prev[:], 0.0)
            for t in range(T):
                x_rhs = combo[H:128, ts(t, BC)]
                h_rhs = combo[0:H, ts(t, BC)]
                p_zr = pp.tile([2 * H, BC], f32, tag="zr")
                nc.tensor.matmul(out=p_zr[:], lhsT=gwx[:, 0 : 2 * H], rhs=x_rhs,
                                 start=True, stop=False)
                nc.tensor.matmul(out=p_zr[:], lhsT=gwh[:, 0 : 2 * H], rhs=h_rhs,
                                 start=False, stop=True)
                p_nx = pp.tile([H, BC], f32, tag="nx")
                nc.tensor.matmul(out=p_nx[:], lhsT=gwx[:, 2 * H : 3 * H], rhs=x_rhs,
                                 start=True, stop=True)
                p_nh = pp.tile([H, BC], f32, tag="nh")
                nc.tensor.matmul(out=p_nh[:], lhsT=gwh[:, 2 * H : 3 * H], rhs=h_rhs,
                                 start=True, stop=True)
                u = work.tile([2 * H, BC], f32, tag="u")
                nc.scalar.activation(u[:], p_zr[:], AF.Sigmoid, bias=gbzr[:, 0:1])
                g = work.tile([H, BC], f32, tag="g")
                nc.vector.tensor_tensor(out=g[:], in0=u[H : 2 * H, :], in1=p_nh[:],
                                        op=OP.mult)
                w = work.tile([H, BC], f32, tag="w")
                nc.vector.tensor_tensor(out=w[:], in0=g[:], in1=p_nx[:], op=OP.add)
                n = work.tile([H, BC], f32, tag="n")
                nc.scalar.activation(n[:], w[:], AF.Tanh, bias=gbn[:, 0:1])
                d = work.tile([H, BC], f32, tag="d")
                nc.vector.tensor_tensor(out=d[:], in0=h_prev[:], in1=n[:],
                                        op=OP.subtract)
                e = work.tile([H, BC], f32, tag="e")
                nc.vector.tensor_tensor(out=e[:], in0=u[0:H, :], in1=d[:],
                                        op=OP.mult)
                h_new = work.tile([H, BC], f32, tag="hstate")
                nc.vector.tensor_tensor(out=h_new[:], in0=n[:], in1=e[:], op=OP.add)
                nc.vector.tensor_copy(combo[0:H, ts(t + 1, BC)], h_new[:])
                h_prev = h_new

            # --- aux MLP over (h_t, beh_{t+1}) pairs: combo slots 1..199 ---
            aux_chunks = []
            s = 1
            while s <= T - 1:
                n_sl = min(4, T - s)
                aux_chunks.append((s, n_sl))
                s += n_sl
            for (s0, n_sl) in aux_chunks:
                N = n_sl * BC
                p_a1 = pp1.tile([100, 4 * BC], f32, tag="aux")
                nc.tensor.matmul(out=p_a1[:, 0:N], lhsT=xw1[:],
                                 rhs=combo[:, s0 * BC : (s0 + n_sl) * BC],
                                 start=True, stop=True)
                a1 = work.tile([100, 4 * BC], bf16, tag="a1")
                nc.scalar.activation(a1[:, 0:N], p_a1[:, 0:N], AF.Sigmoid,
                                     bias=xb1[:, 0:1])
                p_a2 = pp1.tile([50, 4 * BC], f32, tag="aux")
                nc.tensor.matmul(out=p_a2[:, 0:N], lhsT=xw2[:], rhs=a1[:, 0:N],
                                 start=True, stop=True)
                a2 = work.tile([50, 4 * BC], bf16, tag="a2")
                nc.scalar.activation(a2[:, 0:N], p_a2[:, 0:N], AF.Sigmoid,
                                     bias=xb2[:, 0:1])
                p_d = pp1.tile([1, 4 * BC], f32, tag="aux")
                nc.tensor.matmul(out=p_d[:, 0:N], lhsT=xw3d[:], rhs=a2[:, 0:N],
                                 start=True, stop=True)
                pclick = work.tile([1, 4 * BC], f32, tag="pclick")
                nc.scalar.activation(pclick[:, 0:N], p_d[:, 0:N], AF.Sigmoid,
                                     bias=xb3d[:, 0:1])
                nc.sync.dma_start(
                    auxp_o[:, (s0 - 1) * BC : (s0 - 1 + n_sl) * BC],
                    pclick[:, 0:N])

            # --- hs (B-layout) for attention ---
            hs = big.tile([BC, T * H], bf16)
            for t in range(T):
                nc.sync.dma_start_transpose(
                    hs[:, t * H : (t + 1) * H], combo[0:H, ts(t + 1, BC)])

            # --- attention: scores, softmax ---
            scores = small.tile([BC, T], f32)
            ACH = 50
            for c in range(T // ACH):
                tmp = work.tile([BC, ACH * H], f32, tag="atmp")
                hs_v = hs[:, c * ACH * H : (c + 1) * ACH * H].rearrange(
                    "b (t h) -> b t h", h=H)
                tg_b = targ[:].rearrange("b (o d) -> b o d", o=1).to_broadcast([BC, ACH, H])
                nc.vector.tensor_tensor(
                    out=tmp[:].rearrange("b (t h) -> b t h", h=H),
                    in0=hs_v, in1=tg_b, op=OP.mult)
                nc.vector.tensor_reduce(
                    out=scores[:, c * ACH : (c + 1) * ACH],
                    in_=tmp[:].rearrange("b (t h) -> b t h", h=H),
                    axis=mybir.AxisListType.X, op=OP.add)
            smax = small.tile([BC, 1], f32)
            nc.vector.tensor_reduce(out=smax[:], in_=scores[:],
                                    axis=mybir.AxisListType.X, op=OP.max)
            nmax = small.tile([BC, 1], f32)
            nc.vector.tensor_scalar_mul(nmax[:], smax[:], -1.0)
            att_e = small.tile([BC, T], f32)
            nc.scalar.activation(att_e[:], scores[:], AF.Exp, bias=nmax[:, 0:1])
            ssum = small.tile([BC, 1], f32)
            nc.vector.tensor_reduce(out=ssum[:], in_=att_e[:],
                                    axis=mybir.AxisListType.X, op=OP.add)
            rinv = small.tile([BC, 1], f32)
            nc.vector.reciprocal(rinv[:], ssum[:])
            att = small.tile([BC, T], f32)
            nc.vector.tensor_scalar(out=att[:], in0=att_e[:],
                                    scalar1=rinv[:, 0:1], scalar2=None,
                                    op0=OP.mult)
            # ones row for folding aug bias into the x-side matmul: combo row 64
            # (behT item-dim 0) is dead after GRU+aux; overwrite with 1.0
            nc.gpsimd.memset(combo[H : H + 1, BC : NSLOT * BC], 1.0)

            # --- AUGRU over t (B-layout; att scale is a per-partition scalar) ---
            h2_prev = work.tile([BC, H], f32, tag="h2state")
            nc.vector.memset(h2_prev[:], 0.0)
            h2t_prev = work.tile([H, BC], bf16, tag="h2t")
            nc.vector.memset(h2t_prev[:], 0.0)
            for t in range(T):
                x_lhsT = combo[0 : H + 1, ts(t + 1, BC)]  # [gru_outT_t; ones]
                p_zr = pp.tile([BC, 2 * H], f32, tag="zr")
                nc.tensor.matmul(out=p_zr[:], lhsT=x_lhsT,
                                 rhs=awx[0 : H + 1, 0 : 2 * H],
                                 start=True, stop=False)
                nc.tensor.matmul(out=p_zr[:], lhsT=h2t_prev[:],
                                 rhs=awh[0:H, 0 : 2 * H], start=False, stop=True)
                p_nx = pp.tile([BC, H], f32, tag="nx")
                nc.tensor.matmul(out=p_nx[:], lhsT=x_lhsT,
                                 rhs=awx[0 : H + 1, 2 * H : 3 * H],
                                 start=True, stop=True)
                p_nh = pp.tile([BC, H], f32, tag="nh")
                nc.tensor.matmul(out=p_nh[:], lhsT=h2t_prev[:],
                                 rhs=awh[0:H, 2 * H : 3 * H],
                                 start=True, stop=True)
                u = work.tile([BC, 2 * H], f32, tag="u2")
                nc.scalar.activation(u[:], p_zr[:], AF.Sigmoid)
                g = work.tile([BC, H], f32, tag="g2")
                nc.vector.tensor_tensor(out=g[:], in0=u[:, H : 2 * H], in1=p_nh[:],
                                        op=OP.mult)
                w = work.tile([BC, H], f32, tag="w2")
                nc.vector.tensor_tensor(out=w[:], in0=g[:], in1=p_nx[:], op=OP.add)
                n = work.tile([BC, H], f32, tag="n2")
                nc.scalar.activation(n[:], w[:], AF.Tanh)
                d = work.tile([BC, H], f32, tag="d2")
                nc.vector.tensor_tensor(out=d[:], in0=n[:], in1=h2_prev[:],
                                        op=OP.subtract)
                e = work.tile([BC, H], f32, tag="e2")
                nc.vector.scalar_tensor_tensor(
                    out=e[:], in0=u[:, 0:H], scalar=att[:, t : t + 1], in1=d[:],
                    op0=OP.mult, op1=OP.mult)
                h2_new = work.tile([BC, H], f32, tag="h2state")
                nc.vector.tensor_tensor(out=h2_new[:], in0=h2_prev[:], in1=e[:],
                                        op=OP.add)
                h2_prev = h2_new
                if t < T - 1:
                    h2b = work.tile([BC, H], bf16, tag="h2b")
                    nc.vector.tensor_copy(h2b[:], h2_new[:])
                    p_tr = pp1.tile([H, BC], bf16, tag="tr")
                    nc.tensor.transpose(out=p_tr[:], in_=h2b[:], identity=ident[:])
                    h2t_new = work.tile([H, BC], bf16, tag="h2t")
                    nc.vector.tensor_copy(h2t_new[:], p_tr[:])
                    h2t_prev = h2t_new

            haug_s = small.tile([BC, H], f32)
            nc.vector.tensor_copy(haug_s[:], h2_prev[:])
            nc.sync.dma_start(haug_o[:], haug_s[:])

    nc.finalize()
    return nc


class _Runner:
    """Compile once, keep a jitted shard_map callable over 8 NeuronCores."""

    def __init__(self, nc, n_cores):
        import jax
        from jax.sharding import Mesh, PartitionSpec, NamedSharding
        from jax.experimental.shard_map import shard_map
        from concourse import bass2jax, mybir

        self.jax, self.bass2jax = jax, bass2jax
        bass2jax.install_neuronx_cc_hook()
        partition_name = (
            nc.partition_id_tensor.name if nc.partition_id_tensor else None
        )
        in_names, out_names, out_avals, zero_outs = [], [], [], []
        for alloc in nc.m.functions[0].allocations:
            if not isinstance(alloc, mybir.MemoryLocationSet):
                continue
            name = alloc.memorylocations[0].name
            if alloc.kind == "ExternalInput":
                if name != partition_name:
                    in_names.append(name)
            elif alloc.kind == "ExternalOutput":
                shape = tuple(alloc.tensor_shape)
                dtype = mybir.dt.np(alloc.dtype)
                out_names.append(name)
                out_avals.append(jax.core.ShapedArray(shape, dtype))
                zero_outs.append(np.zeros(shape, dtype))
        self.in_names, self.out_names = in_names, out_names
        self.out_avals, self.zero_outs = out_avals, zero_outs
        self.n_cores = n_cores
        n_params = len(in_names)
        all_in_names = in_names + out_names + (
            [partition_name] if partition_name else []
        )

        def _body(*args):
            operands = list(args)
            if partition_name:
                operands.append(bass2jax.partition_id_tensor())
            outs = bass2jax._bass_exec_p.bind(
                *operands,
                out_avals=tuple(out_avals),
                in_names=tuple(all_in_names),
                out_names=tuple(out_names),
                lowering_input_output_aliases=(),
                sim_require_finite=True,
                sim_require_nnan=True,
                nc=nc,
            )
            return tuple(outs)

        devices = jax.devices()[:n_cores]
        self.mesh = Mesh(np.asarray(devices), ("core",))
        self.sharding = NamedSharding(self.mesh, PartitionSpec("core"))
        in_specs = (PartitionSpec("core"),) * (n_params + len(out_names))
        out_specs = (PartitionSpec("core"),) * len(out_names)
        self.fn = jax.jit(
            shard_map(_body, mesh=self.mesh, in_specs=in_specs,
                      out_specs=out_specs, check_rep=False)
        )

    def run(self, in_maps):
        n_cores = self.n_cores
        per_core = [[np.asarray(m[n]) for n in self.in_names] for m in in_maps]
        concat_in = [
            np.ascontiguousarray(
                np.concatenate([per_core[c][i] for c in range(n_cores)], axis=0)
            )
            for i in range(len(self.in_names))
        ]
        concat_zeros = [
            np.zeros((n_cores * z.shape[0], *z.shape[1:]), z.dtype)
            for z in self.zero_outs
        ]
        outs = self.fn(*concat_in, *concat_zeros)
        self.jax.block_until_ready(outs)
        return [
            {
                name: np.asarray(outs[i]).reshape(
                    n_cores, *self.out_avals[i].shape
                )[c]
                for i, name in enumerate(self.out_names)
            }
            for c in range(n_cores)
        ]


def _get_runner():
    if "runner" not in _STATE:
        _STATE["runner"] = _Runner(_build(), NCORES)
    return _STATE["runner"]


def _prep_in_maps(inputs):
    g = {k: np.asarray(v) for k, v in inputs.items()}
    item_bf = g["item_emb"].astype(_BF16)
    cate_bf = g["cate_emb"].astype(_BF16)
    user_f = g["user_emb"].astype(np.float32)
    gwx = g["gru_Wx"].astype(_BF16)
    gwh = g["gru_Wh"].astype(_BF16)
    gb = g["gru_b"].astype(np.float32)
    awx = np.vstack([g["aug_Wx"], g["aug_b"][None, :]]).astype(_BF16)
    awh = g["aug_Wh"].astype(_BF16)
    xw1 = g["aux_w1"].astype(_BF16)
    xb1 = g["aux_b1"].astype(np.float32).reshape(100, 1)
    xw2 = g["aux_w2"].astype(_BF16)
    xb2 = g["aux_b2"].astype(np.float32).reshape(50, 1)
    w3 = g["aux_w3"].astype(np.float32)
    b3 = g["aux_b3"].astype(np.float32)
    xw3d = (w3[:, 0] - w3[:, 1]).reshape(50, 1).astype(_BF16)
    xb3d = np.array([[b3[0] - b3[1]]], np.float32)

    in_maps = []
    for c in range(NCORES):
        rs = slice(c * BC, (c + 1) * BC)
        in_maps.append({
            "item_t": item_bf, "cate_t": cate_bf, "user_t": user_f,
            "iidx": g["click_item_ids"][rs].astype(np.int32),
            "cidx": g["click_cate_ids"][rs].astype(np.int32),
            "tidx": g["target_item_id"][rs].astype(np.int32).reshape(BC, 1),
            "tcidx": g["target_cate_id"][rs].astype(np.int32).reshape(BC, 1),
            "uidx": g["user_id"][rs].astype(np.int32).reshape(BC, 1),
            "gwx": gwx, "gwh": gwh,
            "gbzr": gb[0 : 2 * H].reshape(2 * H, 1),
            "gbn": gb[2 * H : 3 * H].reshape(H, 1),
            "awx": awx, "awh": awh,
            "xw1": xw1, "xb1": xb1, "xw2": xw2, "xb2": xb2,
            "xw3d": xw3d, "xb3d": xb3d,
        })
    return in_maps


def kernel(**inputs):
    runner = _get_runner()
    in_maps = _prep_in_maps(inputs)
    res = runner.run(in_maps)

    # host finale: assemble join, BN (cross-batch stats), 3-layer head, softmax
    h_aug = np.concatenate([r["haug"] for r in res], axis=0)        # [B, H]
    user_p = np.concatenate([r["userp"] for r in res], axis=0)      # [B, P]
    join = np.concatenate([h_aug, user_p], axis=1).astype(np.float32)

    g = {k: np.asarray(v) for k, v in inputs.items()}
    mu = join.mean(axis=0)
    var = join.var(axis=0)
    xn = (join - mu) / np.sqrt(var + 1e-3) * g["bn_gamma"] + g["bn_beta"]
    y = np.maximum(xn @ g["fc_w1"] + g["fc_b1"], 0.0)
    y = np.maximum(y @ g["fc_w2"] + g["fc_b2"], 0.0)
    logit = (y @ g["fc_w3"] + g["fc_b3"]).astype(np.float32)
    z = logit - logit.max(axis=-1, keepdims=True)
    ez = np.exp(z)
    output = (ez / ez.sum(axis=-1, keepdims=True)).astype(np.float32)

    pclick = np.concatenate([r["auxp"].reshape(-1) for r in res])
    aux_loss = np.float32(-np.mean(np.log(pclick.astype(np.float64))))

    return output, logit, aux_loss
